# revision 3
# baseline (speedup 1.0000x reference)
"""Trainium2 Bass kernel for nn_Attention (Bahdanau-style attention scoring).

Reference computation (per batch b, source position s):
    cat    = [hidden[b], encoder_outputs[s, b]]            # [4H]
    energy = tanh(attn_w @ cat + attn_b)                   # [H]
    att    = v . energy                                    # scalar
    att    = -1e10 where mask[b, s] == 0
    out[b] = softmax_s(att[b, :])

Distribution: data-parallel over batch B=32 across 8 cores (4 batches/core).
attn_w / attn_b / v are replicated.

Device layout (per core):
    q[b]   = W_h @ hidden[b] + attn_b                        (tiny matmul)
    E      = W_e @ eo[s,b]  via fp32r matmuls, f contracted on partitions
    energy = tanh(E + q)  fused on ACT (bias = per-partition q chunk)
    att    = v . energy   via fp32r mat-vec into PSUM [1, rows]
    softmax over s per b on a [128, BL, S/128] layout (gpsimd cross-partition
    reduces for max/sum).

Host-side prep (sharding/packing only): slice per-core batches, transpose
eo -> [f, b, s] and attn_w -> [f, h] so the contraction dim lands on SBUF
partitions, and pre-chunk attn_b / v to [128, 4].
"""

import os
import sys
from contextlib import ExitStack

import numpy as np

sys.path.insert(0, "/opt/trn_rl_repo")

import concourse.bacc as bacc  # noqa: E402
import concourse.bass as bass  # noqa: E402
import concourse.mybir as mybir  # noqa: E402
import concourse.tile as tile  # noqa: E402
from concourse import bass_isa  # noqa: E402

H = 512
F = 1024          # 2H, per-operand feature width
B = 32
S = 2048
NCORES = 8
BL = B // NCORES  # batches per core

f32 = mybir.dt.float32
f32r = mybir.dt.float32r
i32 = mybir.dt.int32


def build_program(s=S, bl=BL):
    """Build the per-core Bass program (SPMD, no collectives)."""
    fc_n = F // 128         # 8 f-chunks per operand half
    hc_n = H // 128         # 4 h-chunks
    sc_n = s // 512         # row-tiles (of 512 source positions) per batch
    x_n = s // 128          # free width of the [128, bl, x_n] softmax layout

    nc = bacc.Bacc("TRN2", target_bir_lowering=False, debug=False)

    eo_t = nc.dram_tensor("eo_t", [F, bl, s], f32r, kind="ExternalInput")
    hid_t = nc.dram_tensor("hid_t", [F, bl], f32r, kind="ExternalInput")
    mask_d = nc.dram_tensor("mask", [bl, s], i32, kind="ExternalInput")
    w_t = nc.dram_tensor("w_t", [2 * F, H], f32r, kind="ExternalInput")
    b_t = nc.dram_tensor("b_t", [128, hc_n], f32, kind="ExternalInput")
    v_t = nc.dram_tensor("v_t", [128, hc_n], f32r, kind="ExternalInput")
    out_d = nc.dram_tensor("out", [bl, s], f32, kind="ExternalOutput")

    Act = mybir.ActivationFunctionType
    Alu = mybir.AluOpType

    with tile.TileContext(nc) as tc:
        with ExitStack() as ctx:
            const = ctx.enter_context(tc.tile_pool(name="const", bufs=1))
            eop = ctx.enter_context(tc.tile_pool(name="eop", bufs=24))
            enp = ctx.enter_context(tc.tile_pool(name="enp", bufs=8))
            smp = ctx.enter_context(tc.tile_pool(name="smp", bufs=1))
            psmm = ctx.enter_context(
                tc.tile_pool(name="psmm", bufs=6, space=bass.MemorySpace.PSUM)
            )
            psatt = ctx.enter_context(
                tc.tile_pool(name="psatt", bufs=2, space=bass.MemorySpace.PSUM)
            )
            dramp = ctx.enter_context(
                tc.tile_pool(name="dramp", bufs=1, space=bass.MemorySpace.DRAM)
            )

            # ---- constants / small inputs ----
            wT = const.tile([128, 2 * fc_n, H], f32r)
            nc.sync.dma_start(wT[:], w_t[:].rearrange("(fc p) h -> p fc h", p=128))
            hidT = const.tile([128, fc_n, bl], f32r)
            nc.sync.dma_start(hidT[:], hid_t[:].rearrange("(fc p) b -> p fc b", p=128))
            bias = const.tile([128, hc_n], f32)
            nc.sync.dma_start(bias[:], b_t[:])
            vt = const.tile([128, hc_n], f32r)
            nc.sync.dma_start(vt[:], v_t[:])
            maski = const.tile([128, bl, x_n], i32)
            nc.sync.dma_start(maski[:], mask_d[:].rearrange("b (p x) -> p b x", p=128))

            # additive mask: 0 where mask==1, -1e10 where mask==0
            madd = const.tile([128, bl, x_n], f32)
            nc.vector.tensor_copy(madd[:], maski[:])  # int32 -> f32 cast
            nc.vector.tensor_scalar(
                out=madd[:], in0=madd[:], scalar1=1.0, scalar2=1e10,
                op0=Alu.subtract, op1=Alu.mult,
            )

            zb = const.tile([128, 1], f32)
            nc.vector.memset(zb[:], 0.0)

            # ---- q = W_h @ hidden + attn_b  -> [128, hc, b] ----
            qsb = const.tile([128, hc_n, bl], f32)
            for hc in range(hc_n):
                qp = psmm.tile([128, 512], f32, tag="mm")
                for fc in range(fc_n):
                    nc.tensor.matmul(
                        qp[:, :bl],
                        lhsT=wT[:, fc, hc * 128:(hc + 1) * 128],
                        rhs=hidT[:, fc, :],
                        start=(fc == 0),
                        stop=(fc == fc_n - 1),
                    )
                nc.vector.tensor_scalar_add(qsb[:, hc, :], qp[:, :bl], bias[:, hc:hc + 1])

            scratch = dramp.tile([bl, s], f32)

            # ---- main loop over row-tiles (b, sc) of 512 source positions ----
            for b in range(bl):
                for sc in range(sc_n):
                    eot = []
                    for fc in range(fc_n):
                        t = eop.tile([128, 512], f32r, tag="eot")
                        nc.sync.dma_start(
                            t[:], eo_t[fc * 128:(fc + 1) * 128, b, sc * 512:(sc + 1) * 512]
                        )
                        eot.append(t)
                    mm = [
                        psmm.tile([128, 512], f32, tag="mm", name=f"mm{b}_{sc}_{hc}")
                        for hc in range(hc_n)
                    ]
                    for fc in range(fc_n):
                        for hc in range(hc_n):
                            nc.tensor.matmul(
                                mm[hc][:],
                                lhsT=wT[:, fc_n + fc, hc * 128:(hc + 1) * 128],
                                rhs=eot[fc][:],
                                start=(fc == 0),
                                stop=(fc == fc_n - 1),
                            )
                    ap = psatt.tile([1, 512], f32, tag="att")
                    for hc in range(hc_n):
                        en = enp.tile([128, 512], f32r, tag="en")
                        nc.scalar.activation(
                            en[:], mm[hc][:], Act.Tanh, bias=qsb[:, hc, b:b + 1]
                        )
                        nc.tensor.matmul(
                            ap[:],
                            lhsT=vt[:, hc:hc + 1],
                            rhs=en[:],
                            start=(hc == 0),
                            stop=(hc == hc_n - 1),
                        )
                    st = enp.tile([1, 512], f32, tag="attst")
                    nc.scalar.copy(st[:], ap[:])
                    nc.sync.dma_start(scratch[b, sc * 512:(sc + 1) * 512], st[:])

            # ---- masked softmax over s (per b) on [128, bl, x] layout ----
            att4 = smp.tile([128, bl, x_n], f32)
            nc.sync.dma_start(att4[:], scratch[:].rearrange("b (p x) -> p b x", p=128))
            attm = smp.tile([128, bl, x_n], f32)
            nc.vector.tensor_add(attm[:], att4[:], madd[:])
            mx = smp.tile([128, bl], f32)
            nc.vector.reduce_max(mx[:], attm[:], axis=mybir.AxisListType.X)
            mxa = smp.tile([128, bl], f32)
            nc.gpsimd.partition_all_reduce(
                mxa[:], mx[:], channels=128, reduce_op=bass_isa.ReduceOp.max
            )
            ex = smp.tile([128, bl, x_n], f32)
            nc.vector.tensor_sub(
                ex[:], attm[:], mxa[:].unsqueeze(2).to_broadcast([128, bl, x_n])
            )
            nc.scalar.activation(ex[:], ex[:], Act.Exp, bias=zb[:])
            sm = smp.tile([128, bl], f32)
            nc.vector.reduce_sum(sm[:], ex[:], axis=mybir.AxisListType.X)
            sma = smp.tile([128, bl], f32)
            nc.gpsimd.partition_all_reduce(
                sma[:], sm[:], channels=128, reduce_op=bass_isa.ReduceOp.add
            )
            rec = smp.tile([128, bl], f32)
            nc.vector.reciprocal(rec[:], sma[:])
            outv = smp.tile([128, bl, x_n], f32)
            nc.vector.tensor_mul(
                outv[:], ex[:], rec[:].unsqueeze(2).to_broadcast([128, bl, x_n])
            )
            nc.sync.dma_start(out_d[:].rearrange("b (p x) -> p b x", p=128), outv[:])

    nc.compile()
    return nc


def round_fp32r(a):
    """Round fp32 to the PE's FP32r encoding (12-bit significand, RN-up)."""
    u = np.ascontiguousarray(a, dtype=np.float32).view(np.uint32)
    r = ((u + 0x800) & 0xFFFFF000).astype(np.uint32)
    return r.view(np.float32)


def make_in_maps(hidden, encoder_outputs, mask, attn_w, attn_b, v, s=S, bl=BL,
                 ncores=NCORES):
    """Host-side shard + pack: per-core input dicts."""
    hc_n = H // 128
    w_t = round_fp32r(attn_w.T)                               # [2F, H]
    b_t = np.ascontiguousarray(attn_b.reshape(hc_n, 128).T)   # [128, hc]
    v_t = round_fp32r(v.reshape(hc_n, 128).T)                 # [128, hc]
    in_maps = []
    for c in range(ncores):
        bsl = slice(c * bl, (c + 1) * bl)
        eo_c = encoder_outputs[:, bsl, :]                      # [s, bl, F]
        in_maps.append({
            "eo_t": round_fp32r(eo_c.transpose(2, 1, 0)),            # [F, bl, s]
            "hid_t": round_fp32r(hidden[bsl].T),                     # [F, bl]
            "mask": np.ascontiguousarray(mask[bsl]),                 # [bl, s]
            "w_t": w_t,
            "b_t": b_t,
            "v_t": v_t,
        })
    return in_maps


_cached_nc = None


def kernel(hidden, encoder_outputs, mask, attn_w, attn_b, v):
    from concourse.bass_utils import run_bass_kernel_spmd

    global _cached_nc
    hidden = np.asarray(hidden, dtype=np.float32)
    encoder_outputs = np.asarray(encoder_outputs, dtype=np.float32)
    mask = np.asarray(mask)
    attn_w = np.asarray(attn_w, dtype=np.float32)
    attn_b = np.asarray(attn_b, dtype=np.float32)
    v = np.asarray(v, dtype=np.float32)

    if _cached_nc is None:
        _cached_nc = build_program()
    nc = _cached_nc

    in_maps = make_in_maps(hidden, encoder_outputs, mask, attn_w, attn_b, v)
    res = run_bass_kernel_spmd(nc, in_maps, core_ids=list(range(NCORES)))
    if res.exec_time_ns is not None:
        print(f"HW exec time: {res.exec_time_ns} ns")
        trace = res.instructions_and_trace
        if trace is not None:
            print(f"trace: {trace[1]}")
    out = np.concatenate([r["out"] for r in res.results], axis=0)
    return out.astype(np.float32)


if __name__ == "__main__":
    # smoke test against locally generated random inputs
    rng = np.random.default_rng(0)
    hid = rng.standard_normal((B, 2 * H), dtype=np.float32)
    eo = rng.standard_normal((S, B, 2 * H), dtype=np.float32)
    msk = rng.integers(0, 2, size=(B, S)).astype(np.int32)
    bound = 1.0 / np.sqrt(4 * H)
    aw = rng.uniform(-bound, bound, size=(H, 4 * H)).astype(np.float32)
    ab = rng.uniform(-bound, bound, size=(H,)).astype(np.float32)
    vv = rng.random(H, dtype=np.float32)
    out = kernel(hid, eo, msk, aw, ab, vv)
    print(out.shape, out.dtype, out.sum(axis=1)[:4])
